# revision 1
# baseline (speedup 1.0000x reference)
"""MD-RNN (4-direction 2D GRU) Trainium2 kernel.

Sharding: 8-way data-parallel over batch (B=256 -> 32 per core); each core runs
all 4 directional 2D-GRU scans as anti-diagonal wavefronts, interleaved so the
tensor engine stays busy while other engines run the gate nonlinearities.

Layout ("transposed" / hidden-on-partition):
  - hidden states stored as h^T tiles: (128 partitions = hidden chunk, cells*B free)
  - per cell, psum accumulates gs^T = Wx_aug^T@patch_aug + Uh^T@h_above + Uh2^T@h_left
    (r,z gate chunks include the input projection + bias; the n-chunk input
    projection goes to a separate psum tile since it is not multiplied by r)
  - gate math on vector/scalar engines in the same transposed layout; the new
    h^T tile directly feeds the next diagonal's matmuls (no transposes anywhere).

The patch tensor (im2col of x, plus a constant-ones row for the bias trick) is
built host-side (pure data movement) and streamed per-diagonal from DRAM.
"""

import numpy as np
import ml_dtypes

GRID = 4
N_IMG = 32
S = N_IMG - (GRID - 1)          # 29 patch positions per axis
B_FULL = 256
N_CORES = 8
B = B_FULL // N_CORES           # 32 batch per core
H = 256
H3 = 3 * H                      # 768
OUT_DIM = 10
K_IN = GRID * GRID + 1          # 16 patch elems + ones row (bias trick)

FWD = list(range(S))                 # 29 entries
BWD = list(range(S - 2, -1, -1))     # 28 entries (reference off-by-one kept)
DIRS = [(FWD, FWD), (BWD, FWD), (FWD, BWD), (BWD, BWD)]

CELLS_PER_CHUNK = 16            # 16 cells * B=32 = 512 = one psum bank (fp32)

# Recurrence matmul/storage dtype: "bf16" or "f32" (f32 storage + float32r matmuls)
RD_MODE = "bf16"
GX_F32R = False
REPEAT = 1                      # body repetitions (timing calibration only)                  # input-projection matmul as float32r (full rate)


def _diag_infos():
    """Per direction: list over diagonals of (i_lo, i_hi, global cell base)."""
    infos = []
    base = 0
    for (yi, xi) in DIRS:
        ny, nx = len(yi), len(xi)
        diags = []
        for d in range(ny + nx - 1):
            ilo = max(0, d - (nx - 1))
            ihi = min(d, ny - 1)
            diags.append((ilo, ihi, base))
            base += ihi - ilo + 1
        infos.append(diags)
    return infos, base


DIAG_INFOS, TOT_CELLS = _diag_infos()


def _scan_index_arrays():
    """Image-space (y, x) of every cell in pt order (dir-major, diag-major)."""
    ys, xs = [], []
    for a, (yi, xi) in enumerate(DIRS):
        ny, nx = len(yi), len(xi)
        for d, (ilo, ihi, _) in enumerate(DIAG_INFOS[a]):
            for i in range(ilo, ihi + 1):
                ys.append(yi[i])
                xs.append(xi[d - i])
    return np.asarray(ys), np.asarray(xs)


YS, XS = _scan_index_arrays()


def _chunk_sizes(k):
    nch = (k + CELLS_PER_CHUNK - 1) // CELLS_PER_CHUNK
    lo = k // nch
    rem = k - lo * nch
    return [lo + 1] * rem + [lo] * (nch - rem)


def make_pt(xc):
    """(B, 32, 32) core batch slice -> (17, TOT_CELLS*B) float32 patch matrix."""
    from numpy.lib.stride_tricks import sliding_window_view
    w = sliding_window_view(xc, (GRID, GRID), axis=(1, 2))   # (B, 29, 29, 4, 4)
    p = w[:, YS, XS].reshape(xc.shape[0], TOT_CELLS, GRID * GRID)  # (B, T, 16)
    p = np.ascontiguousarray(p.transpose(2, 1, 0)).reshape(GRID * GRID, -1)
    ones = np.ones((1, p.shape[1]), np.float32)
    pt = np.concatenate([p, ones], axis=0)
    return np.ascontiguousarray(_np_rd(pt))


def _np_rd(x):
    return x.astype(ml_dtypes.bfloat16) if RD_MODE == "bf16" else x.astype(np.float32)


def make_weight_maps(Wx, Uh, Uh2, b, W_out, b_out):
    Wx, Uh, Uh2 = (np.asarray(t, np.float32) for t in (Wx, Uh, Uh2))
    b, W_out, b_out = (np.asarray(t, np.float32) for t in (b, W_out, b_out))
    uh = np.empty((4, 2, 128, 2 * H3), np.float32)
    for a in range(4):
        for kc in range(2):
            uh[a, kc, :, :H3] = Uh[a][kc * 128:(kc + 1) * 128]
            uh[a, kc, :, H3:] = Uh2[a][kc * 128:(kc + 1) * 128]
    wxa = np.empty((4, K_IN, H3), np.float32)
    for a in range(4):
        wxa[a, :GRID * GRID] = Wx[a]
        wxa[a, GRID * GRID] = b[a]
    wo = np.ascontiguousarray(W_out.reshape(8, 128, OUT_DIM))
    bo = np.ascontiguousarray(b_out.reshape(1, OUT_DIM))
    return {
        "uh": _np_rd(uh),
        "wxa": _np_rd(wxa),
        "wo": wo,
        "bo": bo,
    }


def _build_nc():
    import concourse.bacc as bacc
    import concourse.mybir as mybir
    import concourse.tile as tile

    f32 = mybir.dt.float32
    f32r = mybir.dt.float32r
    RD = mybir.dt.bfloat16 if RD_MODE == "bf16" else f32
    AF = mybir.ActivationFunctionType
    ALU = mybir.AluOpType

    nc = bacc.Bacc("TRN2", target_bir_lowering=False, debug=False,
                   num_devices=N_CORES)
    pt_d = nc.dram_tensor("pt", [K_IN, TOT_CELLS * B], RD, kind="ExternalInput")
    uh_d = nc.dram_tensor("uh", [4, 2, 128, 2 * H3], RD, kind="ExternalInput")
    wxa_d = nc.dram_tensor("wxa", [4, K_IN, H3], RD, kind="ExternalInput")
    wo_d = nc.dram_tensor("wo", [8, 128, OUT_DIM], f32, kind="ExternalInput")
    bo_d = nc.dram_tensor("bo", [1, OUT_DIM], f32, kind="ExternalInput")
    out_d = nc.dram_tensor("out", [B, OUT_DIM], f32, kind="ExternalOutput")

    with tile.TileContext(nc) as tc:
        from contextlib import ExitStack
        with ExitStack() as ctx:
            const = ctx.enter_context(tc.tile_pool(name="const", bufs=1))
            ptp = ctx.enter_context(tc.tile_pool(name="ptp", bufs=6))
            ps = ctx.enter_context(tc.tile_pool(name="ps", bufs=8, space="PSUM"))
            hps = [ctx.enter_context(tc.tile_pool(name=f"h{a}", bufs=3))
                   for a in range(4)]
            ew = ctx.enter_context(tc.tile_pool(name="ew", bufs=5))
            hd = ctx.enter_context(tc.tile_pool(name="hd", bufs=1))

            # --- resident weights ---
            uh_sb = {}
            for a in range(4):
                for kc in range(2):
                    t = const.tile([128, 2 * H3], RD, tag=f"uh{a}{kc}")
                    nc.sync.dma_start(out=t, in_=uh_d[a, kc])
                    uh_sb[a, kc] = t
            wxa_sb = {}
            for a in range(4):
                t = const.tile([K_IN, H3], RD, tag=f"wxa{a}")
                nc.sync.dma_start(out=t, in_=wxa_d[a])
                wxa_sb[a] = t
            wo_sb = const.tile([128, 8 * OUT_DIM], f32, tag="wo")
            for c in range(8):
                nc.sync.dma_start(out=wo_sb[:, c * OUT_DIM:(c + 1) * OUT_DIM],
                                  in_=wo_d[c])
            bo_sb = const.tile([1, OUT_DIM], f32, tag="bo")
            nc.sync.dma_start(out=bo_sb, in_=bo_d[:, :])
            ones_sb = const.tile([1, B], f32, tag="ones")
            nc.vector.memset(ones_sb, 1.0)
            zero_h = const.tile([128, 2, 2 * B], RD, tag="zeroh")
            nc.vector.memset(zero_h, 0.0)

            def emit_chunk(a, prev_t, s_a, cbase, c0, c1, ht):
                fd = (c1 - c0) * B
                ptt = ptp.tile([K_IN, CELLS_PER_CHUNK * B], RD, tag="pt")
                nc.sync.dma_start(
                    out=ptt[:, :fd],
                    in_=pt_d[:, (cbase + c0) * B:(cbase + c1) * B])
                above = {kc: prev_t[:, kc, (s_a + c0) * B:(s_a + c1) * B]
                         for kc in (0, 1)}
                left = {kc: prev_t[:, kc, (s_a + 1 + c0) * B:(s_a + 1 + c1) * B]
                        for kc in (0, 1)}

                gate = [None] * 6
                xnb = [None] * 2
                for mc in range(6):
                    pst = ps.tile([128, CELLS_PER_CHUNK * B], f32, tag="g")
                    po = pst[:, :fd]
                    wx_l = wxa_sb[a][:, mc * 128:(mc + 1) * 128]
                    pt_r = ptt[:, :fd]
                    if GX_F32R:
                        wx_l = wx_l.bitcast(f32r)
                        pt_r = pt_r.bitcast(f32r)
                    uh_mm = []
                    for kc in (0, 1):
                        lu = uh_sb[a, kc][:, mc * 128:(mc + 1) * 128]
                        lu2 = uh_sb[a, kc][:, H3 + mc * 128:H3 + (mc + 1) * 128]
                        uh_mm.append((lu, above[kc]))
                        uh_mm.append((lu2, left[kc]))
                    if mc < 4:
                        nc.tensor.matmul(po, wx_l, pt_r, start=True, stop=False)
                        for q, (lhs, rhs) in enumerate(uh_mm):
                            nc.tensor.matmul(po, lhs, rhs, start=False,
                                             stop=(q == 3))
                        gate[mc] = pst
                    else:
                        for q, (lhs, rhs) in enumerate(uh_mm):
                            nc.tensor.matmul(po, lhs, rhs, start=(q == 0),
                                             stop=(q == 3))
                        gate[mc] = pst
                        xt = ps.tile([128, CELLS_PER_CHUNK * B], f32, tag="g")
                        nc.tensor.matmul(xt[:, :fd], wx_l, pt_r,
                                         start=True, stop=True)
                        xnb[mc - 4] = xt

                for kc in (0, 1):
                    rt = ew.tile([128, CELLS_PER_CHUNK * B], RD, tag="r")
                    nc.scalar.activation(rt[:, :fd], gate[kc][:, :fd], AF.Sigmoid)
                    zt = ew.tile([128, CELLS_PER_CHUNK * B], RD, tag="z")
                    nc.scalar.activation(zt[:, :fd], gate[2 + kc][:, :fd],
                                         AF.Sigmoid)
                    xn = ew.tile([128, CELLS_PER_CHUNK * B], RD, tag="xn")
                    nc.scalar.copy(xn[:, :fd], xnb[kc][:, :fd])
                    t1 = ew.tile([128, CELLS_PER_CHUNK * B], RD, tag="t1")
                    nc.vector.tensor_mul(t1[:, :fd], rt[:, :fd],
                                         gate[4 + kc][:, :fd])
                    t2 = ew.tile([128, CELLS_PER_CHUNK * B], RD, tag="t2")
                    nc.vector.tensor_add(t2[:, :fd], t1[:, :fd], xn[:, :fd])
                    nt = ew.tile([128, CELLS_PER_CHUNK * B], RD, tag="n")
                    nc.scalar.activation(nt[:, :fd], t2[:, :fd], AF.Tanh)
                    st = ew.tile([128, CELLS_PER_CHUNK * B], RD, tag="s")
                    nc.gpsimd.tensor_add(st[:, :fd], above[kc], left[kc])
                    dt_ = ew.tile([128, CELLS_PER_CHUNK * B], RD, tag="d")
                    nc.vector.scalar_tensor_tensor(
                        dt_[:, :fd], st[:, :fd], 0.5, nt[:, :fd],
                        ALU.mult, ALU.subtract)
                    et = ew.tile([128, CELLS_PER_CHUNK * B], RD, tag="e")
                    nc.vector.tensor_mul(et[:, :fd], zt[:, :fd], dt_[:, :fd])
                    nc.gpsimd.tensor_add(ht[:, kc, (1 + c0) * B:(1 + c1) * B],
                                         et[:, :fd], nt[:, :fd])

            # --- main wavefront, 4 directions interleaved per diagonal ---
            max_nd = max(len(di) for di in DIAG_INFOS)
            for _rep in range(REPEAT):
              h_prev = {a: None for a in range(4)}
              for d in range(max_nd):
                 for a in range(4):
                    if d >= len(DIAG_INFOS[a]):
                        continue
                    ilo, ihi, cbase = DIAG_INFOS[a][d]
                    k = ihi - ilo + 1
                    ht = hps[a].tile([128, 2, (k + 2) * B], RD, tag=f"h{a}")
                    nc.gpsimd.memset(ht[:, :, 0:B], 0.0)
                    nc.gpsimd.memset(ht[:, :, (k + 1) * B:(k + 2) * B], 0.0)
                    if d == 0:
                        prev_t, k_prev, ilo_prev = zero_h, 0, 0
                    else:
                        prev_t, k_prev, ilo_prev = h_prev[a]
                    s_a = ilo - ilo_prev
                    assert 0 <= s_a and s_a + k <= k_prev + 2, (a, d)
                    c0 = 0
                    for cs in _chunk_sizes(k):
                        emit_chunk(a, prev_t, s_a, cbase, c0, c0 + cs, ht)
                        c0 += cs
                    h_prev[a] = (ht, k, ilo)

            # --- head: logits = hcat @ W_out + b_out ; log_softmax ---
            hfin = []
            for a in range(4):
                ht, k, _ = h_prev[a]
                assert k == 1
                for kc in (0, 1):
                    t = hd.tile([128, B], f32, tag=f"hf{a}{kc}")
                    nc.scalar.copy(t, ht[:, kc, B:2 * B])
                    hfin.append(t)
            pl_t = ps.tile([128, CELLS_PER_CHUNK * B], f32, tag="g")
            pl = pl_t[:B, :OUT_DIM]
            for c, t in enumerate(hfin):
                nc.tensor.matmul(pl, t, wo_sb[:, c * OUT_DIM:(c + 1) * OUT_DIM],
                                 start=(c == 0), stop=False)
            nc.tensor.matmul(pl, ones_sb[:1, :B], bo_sb, start=False, stop=True)
            mx = hd.tile([B, 1], f32, tag="mx")
            nc.vector.reduce_max(mx, pl, axis=mybir.AxisListType.X)
            nmx = hd.tile([B, 1], f32, tag="nmx")
            nc.vector.tensor_scalar_mul(nmx, mx, -1.0)
            exv = hd.tile([B, OUT_DIM], f32, tag="exv")
            nc.scalar.activation(exv, pl, AF.Exp, bias=nmx, scale=1.0)
            sm = hd.tile([B, 1], f32, tag="sm")
            nc.vector.reduce_sum(sm, exv, axis=mybir.AxisListType.X)
            lnz = hd.tile([B, 1], f32, tag="lnz")
            nc.scalar.activation(lnz, sm, AF.Ln)
            tot = hd.tile([B, 1], f32, tag="tot")
            nc.vector.tensor_add(tot, lnz, mx)
            ntot = hd.tile([B, 1], f32, tag="ntot")
            nc.vector.tensor_scalar_mul(ntot, tot, -1.0)
            ot = hd.tile([B, OUT_DIM], f32, tag="ot")
            nc.scalar.activation(ot, pl, AF.Identity, bias=ntot, scale=1.0)
            nc.sync.dma_start(out=out_d[:, :], in_=ot)

    nc.compile()
    return nc


_CACHE = {}


def get_nc():
    if "nc" not in _CACHE:
        _CACHE["nc"] = _build_nc()
    return _CACHE["nc"]


def make_in_maps(x, Wx, Uh, Uh2, b, W_out, b_out):
    x = np.asarray(x, np.float32)
    wm = make_weight_maps(Wx, Uh, Uh2, b, W_out, b_out)
    in_maps = []
    for c in range(N_CORES):
        xc = x[c * B:(c + 1) * B]
        m = dict(wm)
        m["pt"] = make_pt(xc)
        in_maps.append(m)
    return in_maps


def kernel(x, Wx, Uh, Uh2, b, W_out, b_out):
    from concourse.bass_utils import run_bass_kernel_spmd
    nc = get_nc()
    in_maps = make_in_maps(x, Wx, Uh, Uh2, b, W_out, b_out)
    res = run_bass_kernel_spmd(nc, in_maps, list(range(N_CORES)))
    out = np.concatenate([res.results[c]["out"] for c in range(N_CORES)], axis=0)
    return out.astype(np.float32)



# revision 2
# speedup vs baseline: 1.2480x; 1.2480x over previous
"""MD-RNN (4-direction 2D GRU) Trainium2 kernel, v4.

Sharding: 8 cores = 2 direction-groups x 4 batch-quarters. Cores 0-3 run
directions {0,3}, cores 4-7 run {1,2}, each on a 64-row batch quarter.
Doubling B per (dir, diagonal) cuts the matmul-instruction count lost to
diagonal-boundary quantization (7800 vs 9540 MMs/core) -- the kernel is PE
issue-rate bound, so instructions ~= time.

To keep one SPMD program for both groups, the shorter directions (1 and 3,
which have ny=28) are padded with a dummy FIRST row of all-zero patch
columns: with zero patch and zero neighbors the GRU cell computes exactly
h=0 (sigma(0)=0.5, tanh(0)=0 -> h' = 0), i.e. the dummy row reproduces the
virtual zero boundary row, so every real cell sees identical inputs. Both
groups then share slot tables [(29,29), (29,28)] and 1653 cells.

Each core emits PARTIAL logits (its 2 directions' h_final @ W_out blocks);
the host sums partner-core partials, adds b_out, applies log_softmax
(256x10 flops, negligible).

Carries over from v2/v3: persistent ping-pong h rows with permanent zero
margins (no per-diagonal memsets), gate math fused over both hidden chunks
([128, 2, fd] ops), psum as 4 tags x 2 banks, row-tiled K=17 Wx matmuls.
"""

import numpy as np
import ml_dtypes

GRID = 4
N_IMG = 32
S = N_IMG - (GRID - 1)          # 29 patch positions per axis
B_FULL = 256
N_CORES = 8
B = 64                          # batch rows per core (quarter)
H = 256
H3 = 3 * H
OUT_DIM = 10
K_IN = GRID * GRID + 1          # 16 patch elems + ones row (bias trick)
NPOS = S + 1                    # h-row positions: p=0 margin, p=i+1 for row i

FWD = list(range(S))                 # 29 entries
BWD = list(range(S - 2, -1, -1))     # 28 entries (reference off-by-one kept)
DIRS = [(FWD, FWD), (BWD, FWD), (FWD, BWD), (BWD, BWD)]
GROUP_SLOT_DIRS = [(0, 3), (1, 2)]   # per group: dirs of slot 0, slot 1
SLOT_SHAPES = [(S, S), (S, S - 1)]   # padded (ny, nx) per slot

CELLS_PER_CHUNK = 8             # 8 cells * B=64 = 512 cols = one psum bank

ROW_TILE_WX = True
REPEAT = 1


def _slot_tables():
    tabs = []
    base = 0
    for (ny, nx) in SLOT_SHAPES:
        diags = []
        for d in range(ny + nx - 1):
            ilo = max(0, d - (nx - 1))
            ihi = min(d, ny - 1)
            diags.append((ilo, ihi, base))
            base += ihi - ilo + 1
        tabs.append(diags)
    return tabs, base


TABS, TOT_CELLS = _slot_tables()


def _scan_index_arrays(grp):
    """Per pt column (slot-major, diag-major): image (y, x), or (-1,-1) for
    dummy padded cells."""
    ys, xs = [], []
    for j, (ny, nx) in enumerate(SLOT_SHAPES):
        a = GROUP_SLOT_DIRS[grp][j]
        yi, xi = DIRS[a]
        pad = ny - len(yi)
        assert nx == len(xi) and pad in (0, 1)
        for d, (ilo, ihi, _) in enumerate(TABS[j]):
            for i in range(ilo, ihi + 1):
                if i < pad:
                    ys.append(-1)
                    xs.append(-1)
                else:
                    ys.append(yi[i - pad])
                    xs.append(xi[d - i])
    return np.asarray(ys), np.asarray(xs)


GROUP_YX = [_scan_index_arrays(0), _scan_index_arrays(1)]


def _chunk_sizes(k):
    nch = (k + CELLS_PER_CHUNK - 1) // CELLS_PER_CHUNK
    lo = k // nch
    rem = k - lo * nch
    return [lo + 1] * rem + [lo] * (nch - rem)


def make_pt(xc, grp):
    """(B, 32, 32) batch slice -> (17, TOT_CELLS*B) bf16 patches, grp order.
    Dummy cells (padding rows) get all-zero columns (including ones row)."""
    from numpy.lib.stride_tricks import sliding_window_view
    ys, xs = GROUP_YX[grp]
    w = sliding_window_view(xc, (GRID, GRID), axis=(1, 2))   # (B,29,29,4,4)
    dummy = ys < 0
    p = w[:, np.where(dummy, 0, ys), np.where(dummy, 0, xs)]
    p = p.reshape(xc.shape[0], TOT_CELLS, GRID * GRID)       # (B, T, 16)
    ones = np.broadcast_to(
        (~dummy)[None, :, None], (xc.shape[0], TOT_CELLS, 1))
    p = np.concatenate([p * (~dummy)[None, :, None], ones], axis=2)
    p = np.ascontiguousarray(
        p.transpose(2, 1, 0)).reshape(K_IN, TOT_CELLS * xc.shape[0])
    return np.ascontiguousarray(p.astype(ml_dtypes.bfloat16))


def make_weight_maps(Wx, Uh, Uh2, b, W_out, b_out):
    Wx, Uh, Uh2 = (np.asarray(t, np.float32) for t in (Wx, Uh, Uh2))
    b, W_out = np.asarray(b, np.float32), np.asarray(W_out, np.float32)
    KW = 113 if ROW_TILE_WX else K_IN
    maps = []
    for grp in range(2):
        uh = np.empty((2, 128, 2, 2 * H3), np.float32)
        wxa = np.zeros((2, KW, H3), np.float32)
        wo = np.empty((2, 2, 128, OUT_DIM), np.float32)
        for j in range(2):
            a = GROUP_SLOT_DIRS[grp][j]
            for kc in range(2):
                uh[j, :, kc, :H3] = Uh[a][kc * 128:(kc + 1) * 128]
                uh[j, :, kc, H3:] = Uh2[a][kc * 128:(kc + 1) * 128]
            reps = range(4) if ROW_TILE_WX else range(1)
            for g in reps:
                wxa[j, 32 * g:32 * g + GRID * GRID] = Wx[a]
                wxa[j, 32 * g + GRID * GRID] = b[a]
            for kc in range(2):
                wo[j, kc] = W_out[a * H + kc * 128:a * H + (kc + 1) * 128]
        maps.append({
            "uh": np.ascontiguousarray(
                uh.astype(ml_dtypes.bfloat16)).reshape(2, 128, 2 * 2 * H3),
            "wxa": np.ascontiguousarray(wxa.astype(ml_dtypes.bfloat16)),
            "wo": np.ascontiguousarray(wo.reshape(4, 128, OUT_DIM)),
        })
    return maps


def _build_nc():
    import concourse.bacc as bacc
    import concourse.mybir as mybir
    import concourse.tile as tile

    f32 = mybir.dt.float32
    bf16 = mybir.dt.bfloat16
    AF = mybir.ActivationFunctionType
    ALU = mybir.AluOpType

    KW = 113 if ROW_TILE_WX else K_IN
    nc = bacc.Bacc("TRN2", target_bir_lowering=False, debug=False,
                   num_devices=N_CORES)
    pt_d = nc.dram_tensor("pt", [K_IN, TOT_CELLS * B], bf16,
                          kind="ExternalInput")
    uh_d = nc.dram_tensor("uh", [2, 128, 2 * 2 * H3], bf16,
                          kind="ExternalInput")
    wxa_d = nc.dram_tensor("wxa", [2, KW, H3], bf16, kind="ExternalInput")
    wo_d = nc.dram_tensor("wo", [4, 128, OUT_DIM], f32, kind="ExternalInput")
    out_d = nc.dram_tensor("out", [B, OUT_DIM], f32, kind="ExternalOutput")

    CB = CELLS_PER_CHUNK * B    # 512

    with tile.TileContext(nc) as tc:
        from contextlib import ExitStack
        with ExitStack() as ctx:
            const = ctx.enter_context(tc.tile_pool(name="const", bufs=1))
            ptp = ctx.enter_context(tc.tile_pool(name="ptp", bufs=6))
            ps = ctx.enter_context(tc.tile_pool(name="ps", bufs=1,
                                                space="PSUM"))
            hst = ctx.enter_context(tc.tile_pool(name="hst", bufs=1))
            ew = ctx.enter_context(tc.tile_pool(name="ew", bufs=3))
            hd = ctx.enter_context(tc.tile_pool(name="hd", bufs=1))

            uh_sb = {}
            for j in range(2):
                t = const.tile([128, 2, 2 * H3], bf16, tag=f"uh{j}")
                nc.sync.dma_start(out=t, in_=uh_d[j])
                uh_sb[j] = t
            wxa_sb = {}
            for j in range(2):
                t = const.tile([KW, H3], bf16, tag=f"wxa{j}")
                nc.sync.dma_start(out=t, in_=wxa_d[j])
                wxa_sb[j] = t
            wo_sb = const.tile([128, 4 * OUT_DIM], f32, tag="wo")
            for c in range(4):
                nc.sync.dma_start(out=wo_sb[:, c * OUT_DIM:(c + 1) * OUT_DIM],
                                  in_=wo_d[c])

            h_bf = {}
            for j in range(2):
                for sl in range(2):
                    t = hst.tile([128, 2, NPOS * B], bf16, tag=f"hb{j}{sl}")
                    nc.vector.memset(t, 0.0)
                    h_bf[j, sl] = t

            def emit_chunk(j, d, ilo, cbase, c0, c1):
                k = c1 - c0
                fd = k * B
                sl = d % 2
                pT_bf = h_bf[j, 1 - sl]
                cT_bf = h_bf[j, sl]
                pos0 = (ilo + c0) * B
                wsl = slice(pos0 + B, pos0 + (k + 1) * B)
                asl = slice(pos0, pos0 + k * B)
                lsl = slice(pos0 + B, pos0 + (k + 1) * B)

                rowtile = ROW_TILE_WX and fd == CB
                ptt = ptp.tile([KW, CB], bf16, tag="pt")
                src = pt_d[:, (cbase + c0) * B:(cbase + c1) * B]
                if rowtile:
                    for g4 in range(4):
                        nc.sync.dma_start(
                            out=ptt[32 * g4:32 * g4 + K_IN, :fd], in_=src)
                else:
                    nc.sync.dma_start(out=ptt[0:K_IN, :fd], in_=src)

                R = ps.tile([128, 2, CB], f32, tag="R")
                Z = ps.tile([128, 2, CB], f32, tag="Z")
                GN = ps.tile([128, 2, CB], f32, tag="GN")
                XN = ps.tile([128, 2, CB], f32, tag="XN")

                def wx_mm(pst, m, mc, start, stop):
                    if rowtile:
                        g4 = mc % 4
                        nc.tensor.matmul(
                            pst[:, m, :fd],
                            wxa_sb[j][32 * g4:32 * g4 + K_IN,
                                      mc * 128:(mc + 1) * 128],
                            ptt[32 * g4:32 * g4 + K_IN, :fd],
                            start=start, stop=stop, tile_position=(32 * g4, 0))
                    else:
                        nc.tensor.matmul(
                            pst[:, m, :fd],
                            wxa_sb[j][0:K_IN, mc * 128:(mc + 1) * 128],
                            ptt[0:K_IN, :fd], start=start, stop=stop)

                if rowtile:
                    wx_mm(R, 0, 0, True, False)
                    wx_mm(R, 1, 1, True, False)
                    wx_mm(Z, 0, 2, True, False)
                    wx_mm(Z, 1, 3, True, False)
                    wx_mm(XN, 0, 4, True, True)
                    wx_mm(XN, 1, 5, True, True)

                above = {kc: pT_bf[:, kc, asl] for kc in (0, 1)}
                left = {kc: pT_bf[:, kc, lsl] for kc in (0, 1)}
                for g, pst in ((0, R), (1, Z), (2, GN)):
                    for m in (0, 1):
                        mc = 2 * g + m
                        po = pst[:, m, :fd]
                        if g < 2 and not rowtile:
                            wx_mm(pst, m, mc, True, False)
                        q = 0
                        for u in (0, 1):
                            for kc in (0, 1):
                                lw = uh_sb[j][:, kc,
                                              u * H3 + mc * 128:
                                              u * H3 + (mc + 1) * 128]
                                rhs = above[kc] if u == 0 else left[kc]
                                nc.tensor.matmul(
                                    po, lw, rhs,
                                    start=(g == 2 and q == 0),
                                    stop=(q == 3))
                                q += 1
                if not rowtile:
                    wx_mm(XN, 0, 4, True, True)
                    wx_mm(XN, 1, 5, True, True)

                rt = ew.tile([128, 2, CB], bf16, tag="rt")
                nc.scalar.activation(rt[:, :, :fd], R[:, :, :fd], AF.Sigmoid)
                zt = ew.tile([128, 2, CB], bf16, tag="zt")
                nc.scalar.activation(zt[:, :, :fd], Z[:, :, :fd], AF.Sigmoid)
                t1 = ew.tile([128, 2, CB], bf16, tag="t1")
                nc.vector.tensor_mul(t1[:, :, :fd], rt[:, :, :fd],
                                     GN[:, :, :fd])
                t2 = ew.tile([128, 2, CB], bf16, tag="t2")
                nc.vector.tensor_add(t2[:, :, :fd], t1[:, :, :fd],
                                     XN[:, :, :fd])
                nt = ew.tile([128, 2, CB], bf16, tag="nt")
                nc.scalar.activation(nt[:, :, :fd], t2[:, :, :fd], AF.Tanh)
                st = ew.tile([128, 2, CB], bf16, tag="st")
                nc.gpsimd.tensor_add(st[:, :, :fd], pT_bf[:, :, asl],
                                     pT_bf[:, :, lsl])
                dt_ = ew.tile([128, 2, CB], bf16, tag="dt")
                nc.vector.scalar_tensor_tensor(
                    dt_[:, :, :fd], st[:, :, :fd], 0.5, nt[:, :, :fd],
                    ALU.mult, ALU.subtract)
                et = ew.tile([128, 2, CB], bf16, tag="et")
                nc.vector.tensor_mul(et[:, :, :fd], zt[:, :, :fd],
                                     dt_[:, :, :fd])
                nc.gpsimd.tensor_add(cT_bf[:, :, wsl], et[:, :, :fd],
                                     nt[:, :, :fd])

            max_nd = max(len(t) for t in TABS)
            for _rep in range(REPEAT):
                for d in range(max_nd):
                    for j in range(2):
                        if d >= len(TABS[j]):
                            continue
                        ilo, ihi, cbase = TABS[j][d]
                        k = ihi - ilo + 1
                        c0 = 0
                        for cs in _chunk_sizes(k):
                            emit_chunk(j, d, ilo, cbase, c0, c0 + cs)
                            c0 += cs

            # --- head: partial logits for this core's 2 dirs ---
            hfin = []
            for j in range(2):
                ny = TABS[j][-1][1] + 1         # final cell row + 1 == S
                sl = (len(TABS[j]) - 1) % 2
                for kc in (0, 1):
                    t = hd.tile([128, B], f32, tag=f"hf{j}{kc}")
                    nc.scalar.copy(t, h_bf[j, sl][:, kc, ny * B:(ny + 1) * B])
                    hfin.append(t)
            pl_t = ps.tile([128, 2, CB], f32, tag="R")
            pl = pl_t[:B, 0, :OUT_DIM]
            for c, t in enumerate(hfin):
                nc.tensor.matmul(pl, t, wo_sb[:, c * OUT_DIM:(c + 1) * OUT_DIM],
                                 start=(c == 0), stop=(c == 3))
            ot = hd.tile([B, OUT_DIM], f32, tag="ot")
            nc.scalar.copy(ot, pl)
            nc.sync.dma_start(out=out_d[:, :], in_=ot)

    nc.compile()
    return nc


_CACHE = {}


def get_nc():
    if "nc" not in _CACHE:
        _CACHE["nc"] = _build_nc()
    return _CACHE["nc"]


def make_in_maps(x, Wx, Uh, Uh2, b, W_out, b_out):
    x = np.asarray(x, np.float32)
    wms = make_weight_maps(Wx, Uh, Uh2, b, W_out, b_out)
    in_maps = []
    for c in range(N_CORES):
        grp = 0 if c < 4 else 1
        q = c % 4
        xc = x[q * B:(q + 1) * B]
        m = dict(wms[grp])
        m["pt"] = make_pt(xc, grp)
        in_maps.append(m)
    return in_maps


def combine(parts, b_out):
    """parts: (8, 64, 10) per-core partial logits -> (256, 10) log_softmax."""
    parts = np.asarray(parts, np.float32)
    logits = (parts[0:4].reshape(B_FULL, OUT_DIM)
              + parts[4:8].reshape(B_FULL, OUT_DIM)
              + np.asarray(b_out, np.float32)[None, :])
    mx = logits.max(axis=-1, keepdims=True)
    lse = np.log(np.exp(logits - mx).sum(axis=-1, keepdims=True)) + mx
    return (logits - lse).astype(np.float32)


def kernel(x, Wx, Uh, Uh2, b, W_out, b_out):
    from concourse.bass_utils import run_bass_kernel_spmd
    nc = get_nc()
    in_maps = make_in_maps(x, Wx, Uh, Uh2, b, W_out, b_out)
    res = run_bass_kernel_spmd(nc, in_maps, list(range(N_CORES)))
    parts = np.stack([res.results[c]["out"] for c in range(N_CORES)], axis=0)
    return combine(parts, b_out)


# revision 3
# speedup vs baseline: 1.3619x; 1.0913x over previous
"""MD-RNN (4-direction 2D GRU) Trainium2 kernel, v5.

Sharding: 8 cores = 4 directions x 2 batch-halves, B=128 per core. Every
direction's grid is padded to 29x29 with dummy FIRST row/column cells whose
patch columns are all-zero: with zero patch and zero neighbors the GRU cell
computes exactly h=0, reproducing the virtual zero boundary, so all real
cells are unaffected and all 8 cores run ONE identical program. B=128 per
(dir, diagonal) minimizes matmul count lost to diagonal-boundary
quantization: 232 chunks x 30 = 6960 matmuls/core (the kernel is PE
issue/stream bound).

Each core emits PARTIAL logits (its direction's h_final @ W_out block); the
host sums the 4 direction-cores per batch half, adds b_out, applies
log_softmax.

Carries over: persistent ping-pong h rows with permanent zero margins, gate
math fused over both hidden chunks ([128, 2, fd] ops), psum 4 tags x 2
banks, row-tiled K=17 Wx matmuls.
"""

import numpy as np
import ml_dtypes

GRID = 4
N_IMG = 32
S = N_IMG - (GRID - 1)          # 29
B_FULL = 256
N_CORES = 8
B = 128                         # batch rows per core (half)
H = 256
H3 = 3 * H
OUT_DIM = 10
K_IN = GRID * GRID + 1
NPOS = S + 1

FWD = list(range(S))
BWD = list(range(S - 2, -1, -1))
DIRS = [(FWD, FWD), (BWD, FWD), (FWD, BWD), (BWD, BWD)]

CELLS_PER_CHUNK = 4             # 4 cells * B=128 = 512 cols = one psum bank

ROW_TILE_WX = True
MM_MODE = "swic"                # "swic" (fp8 SwInterleave, staged rhs) or "bf16"
REPEAT = 1


def _table():
    diags = []
    base = 0
    for d in range(2 * S - 1):
        ilo = max(0, d - (S - 1))
        ihi = min(d, S - 1)
        diags.append((ilo, ihi, base))
        base += ihi - ilo + 1
    return diags, base


TAB, TOT_CELLS = _table()       # 841 cells


def _scan_index_arrays(a):
    """Per pt column (diag-major): image (y, x) or (-1,-1) for dummy cells."""
    yi, xi = DIRS[a]
    pr, pc = S - len(yi), S - len(xi)
    ys, xs = [], []
    for d, (ilo, ihi, _) in enumerate(TAB):
        for i in range(ilo, ihi + 1):
            jx = d - i
            if i < pr or jx < pc:
                ys.append(-1)
                xs.append(-1)
            else:
                ys.append(yi[i - pr])
                xs.append(xi[jx - pc])
    return np.asarray(ys), np.asarray(xs)


DIR_YX = [_scan_index_arrays(a) for a in range(4)]


def _chunk_sizes(k):
    nch = (k + CELLS_PER_CHUNK - 1) // CELLS_PER_CHUNK
    lo = k // nch
    rem = k - lo * nch
    return [lo + 1] * rem + [lo] * (nch - rem)


def make_pt(xc, a):
    """(B, 32, 32) batch half -> (17, TOT_CELLS*B) bf16 patches for dir a."""
    from numpy.lib.stride_tricks import sliding_window_view
    ys, xs = DIR_YX[a]
    w = sliding_window_view(xc, (GRID, GRID), axis=(1, 2))
    dummy = ys < 0
    p = w[:, np.where(dummy, 0, ys), np.where(dummy, 0, xs)]
    p = p.reshape(xc.shape[0], TOT_CELLS, GRID * GRID)
    ones = np.broadcast_to(
        (~dummy)[None, :, None], (xc.shape[0], TOT_CELLS, 1))
    p = np.concatenate([p * (~dummy)[None, :, None], ones], axis=2)
    p = np.ascontiguousarray(
        p.transpose(2, 1, 0)).reshape(K_IN, TOT_CELLS * xc.shape[0])
    return np.ascontiguousarray(p.astype(ml_dtypes.bfloat16))


def make_weight_maps(Wx, Uh, Uh2, b, W_out, b_out):
    Wx, Uh, Uh2 = (np.asarray(t, np.float32) for t in (Wx, Uh, Uh2))
    b, W_out = np.asarray(b, np.float32), np.asarray(W_out, np.float32)
    KW = 113 if ROW_TILE_WX else K_IN
    maps = []
    for a in range(4):
        uh = np.empty((128, 2, 2 * H3), np.float32)
        wxa = np.zeros((KW, H3), np.float32)
        wo = np.empty((2, 128, OUT_DIM), np.float32)
        for kc in range(2):
            uh[:, kc, :H3] = Uh[a][kc * 128:(kc + 1) * 128]
            uh[:, kc, H3:] = Uh2[a][kc * 128:(kc + 1) * 128]
            wo[kc] = W_out[a * H + kc * 128:a * H + (kc + 1) * 128]
        reps = range(4) if ROW_TILE_WX else range(1)
        for g in reps:
            wxa[32 * g:32 * g + GRID * GRID] = Wx[a]
            wxa[32 * g + GRID * GRID] = b[a]
        # SwInterleave layout per (u, mc) block: per partition p the pairs
        # [W0[p, 127], W1[p, 127], W0[p, 126], W1[p, 126], ...] where
        # W0/W1 are the kc0/kc1 k-subtiles of that 128-col weight block.
        swi = np.empty((128, 12, 256), np.float32)
        for u, U in ((0, Uh[a]), (1, Uh2[a])):
            for mc in range(6):
                W = U[:, mc * 128:(mc + 1) * 128]
                W0, W1 = W[0:128], W[128:256]
                inter = np.stack([W0[:, ::-1], W1[:, ::-1]],
                                 axis=-1).reshape(128, 256)
                swi[:, u * 6 + mc] = inter
        maps.append({
            "uhswi": np.ascontiguousarray(
                swi.astype(ml_dtypes.float8_e4m3)).reshape(128, 12 * 256),
            "uh": np.ascontiguousarray(
                uh.astype(ml_dtypes.bfloat16)).reshape(128, 2 * 2 * H3),
            "wxa": np.ascontiguousarray(wxa.astype(ml_dtypes.bfloat16)),
            "wo": np.ascontiguousarray(wo),
        })
    return maps


def _build_nc():
    import concourse.bacc as bacc
    import concourse.mybir as mybir
    import concourse.tile as tile

    f32 = mybir.dt.float32
    bf16 = mybir.dt.bfloat16
    fp8 = mybir.dt.float8e4
    SWI = mybir.MatmulPerfMode.DoubleRowSwInterleave
    AF = mybir.ActivationFunctionType
    ALU = mybir.AluOpType

    KW = 113 if ROW_TILE_WX else K_IN
    nc = bacc.Bacc("TRN2", target_bir_lowering=False, debug=False,
                   num_devices=N_CORES)
    pt_d = nc.dram_tensor("pt", [K_IN, TOT_CELLS * B], bf16,
                          kind="ExternalInput")
    uh_d = nc.dram_tensor("uh", [128, 2 * 2 * H3], bf16, kind="ExternalInput")
    uhswi_d = nc.dram_tensor("uhswi", [128, 12 * 256], fp8,
                             kind="ExternalInput")
    wxa_d = nc.dram_tensor("wxa", [KW, H3], bf16, kind="ExternalInput")
    wo_d = nc.dram_tensor("wo", [2, 128, OUT_DIM], f32, kind="ExternalInput")
    out_d = nc.dram_tensor("out", [B, OUT_DIM], f32, kind="ExternalOutput")

    CB = CELLS_PER_CHUNK * B    # 512

    with tile.TileContext(nc) as tc:
        from contextlib import ExitStack
        with ExitStack() as ctx:
            const = ctx.enter_context(tc.tile_pool(name="const", bufs=1))
            ptp = ctx.enter_context(tc.tile_pool(name="ptp", bufs=6))
            ps = ctx.enter_context(tc.tile_pool(name="ps", bufs=1,
                                                space="PSUM"))
            hst = ctx.enter_context(tc.tile_pool(name="hst", bufs=1))
            ew = ctx.enter_context(tc.tile_pool(name="ew", bufs=3))
            hd = ctx.enter_context(tc.tile_pool(name="hd", bufs=1))

            uh_sb = const.tile([128, 2, 2 * H3], bf16, tag="uh")
            nc.sync.dma_start(out=uh_sb, in_=uh_d[:, :])
            uhswi_sb = const.tile([128, 12 * 256], fp8, tag="uhswi")
            nc.sync.dma_start(out=uhswi_sb, in_=uhswi_d[:, :])
            wxa_sb = const.tile([KW, H3], bf16, tag="wxa")
            nc.sync.dma_start(out=wxa_sb, in_=wxa_d[:, :])
            wo_sb = const.tile([128, 2 * OUT_DIM], f32, tag="wo")
            for c in range(2):
                nc.sync.dma_start(out=wo_sb[:, c * OUT_DIM:(c + 1) * OUT_DIM],
                                  in_=wo_d[c])

            h_bf = {}
            h_e4 = {}
            for sl in range(2):
                t = hst.tile([128, 2, NPOS * B], bf16, tag=f"hb{sl}")
                nc.vector.memset(t, 0.0)
                h_bf[sl] = t


            def emit_chunk(d, ilo, cbase, c0, c1):
                k = c1 - c0
                fd = k * B
                sl = d % 2
                pT_bf = h_bf[1 - sl]
                cT_bf = h_bf[sl]
                pos0 = (ilo + c0) * B
                wsl = slice(pos0 + B, pos0 + (k + 1) * B)
                asl = slice(pos0, pos0 + k * B)
                lsl = slice(pos0 + B, pos0 + (k + 1) * B)

                rowtile = ROW_TILE_WX and fd == CB
                ptt = ptp.tile([KW, CB], bf16, tag="pt")
                src = pt_d[:, (cbase + c0) * B:(cbase + c1) * B]
                if rowtile:
                    for g4 in range(4):
                        nc.sync.dma_start(
                            out=ptt[32 * g4:32 * g4 + K_IN, :fd], in_=src)
                else:
                    nc.sync.dma_start(out=ptt[0:K_IN, :fd], in_=src)

                R = ps.tile([128, 2, CB], f32, tag="R")
                Z = ps.tile([128, 2, CB], f32, tag="Z")
                GN = ps.tile([128, 2, CB], f32, tag="GN")
                XN = ps.tile([128, 2, CB], f32, tag="XN")

                def wx_mm(pst, m, mc, start, stop):
                    if rowtile:
                        g4 = mc % 4
                        nc.tensor.matmul(
                            pst[:, m, :fd],
                            wxa_sb[32 * g4:32 * g4 + K_IN,
                                   mc * 128:(mc + 1) * 128],
                            ptt[32 * g4:32 * g4 + K_IN, :fd],
                            start=start, stop=stop, tile_position=(32 * g4, 0))
                    else:
                        nc.tensor.matmul(
                            pst[:, m, :fd],
                            wxa_sb[0:K_IN, mc * 128:(mc + 1) * 128],
                            ptt[0:K_IN, :fd], start=start, stop=stop)

                if rowtile:
                    wx_mm(R, 0, 0, True, False)
                    wx_mm(R, 1, 1, True, False)
                    wx_mm(Z, 0, 2, True, False)
                    wx_mm(Z, 1, 3, True, False)
                    wx_mm(XN, 0, 4, True, True)
                    wx_mm(XN, 1, 5, True, True)

                if MM_MODE == "swic":
                    ab8 = ew.tile([128, 2, CB], fp8, tag="ab8")
                    nc.scalar.copy(ab8[:, :, :fd], pT_bf[:, :, asl])
                    lf8 = ew.tile([128, 2, CB], fp8, tag="lf8")
                    nc.scalar.copy(lf8[:, :, :fd], pT_bf[:, :, lsl])
                    above8 = ab8[:, :, :fd]
                    left8 = lf8[:, :, :fd]
                    for g, pst in ((0, R), (1, Z), (2, GN)):
                        for m in (0, 1):
                            mc = 2 * g + m
                            po = pst[:, m, :fd]
                            if g < 2 and not rowtile:
                                wx_mm(pst, m, mc, True, False)
                            for u in (0, 1):
                                blk = u * 6 + mc
                                lw = uhswi_sb[:, blk * 256:(blk + 1) * 256]
                                rhs = above8 if u == 0 else left8
                                nc.tensor.matmul(
                                    po, lw, rhs,
                                    start=(g == 2 and u == 0),
                                    stop=(u == 1), perf_mode=SWI)
                else:
                    above = {kc: pT_bf[:, kc, asl] for kc in (0, 1)}
                    left = {kc: pT_bf[:, kc, lsl] for kc in (0, 1)}
                    for g, pst in ((0, R), (1, Z), (2, GN)):
                        for m in (0, 1):
                            mc = 2 * g + m
                            po = pst[:, m, :fd]
                            if g < 2 and not rowtile:
                                wx_mm(pst, m, mc, True, False)
                            q = 0
                            for u in (0, 1):
                                for kc in (0, 1):
                                    lw = uh_sb[:, kc,
                                               u * H3 + mc * 128:
                                               u * H3 + (mc + 1) * 128]
                                    rhs = above[kc] if u == 0 else left[kc]
                                    nc.tensor.matmul(
                                        po, lw, rhs,
                                        start=(g == 2 and q == 0),
                                        stop=(q == 3))
                                    q += 1
                if not rowtile:
                    wx_mm(XN, 0, 4, True, True)
                    wx_mm(XN, 1, 5, True, True)

                rt = ew.tile([128, 2, CB], bf16, tag="rt")
                nc.scalar.activation(rt[:, :, :fd], R[:, :, :fd], AF.Sigmoid)
                zt = ew.tile([128, 2, CB], bf16, tag="zt")
                nc.scalar.activation(zt[:, :, :fd], Z[:, :, :fd], AF.Sigmoid)
                t1 = ew.tile([128, 2, CB], bf16, tag="t1")
                nc.vector.tensor_mul(t1[:, :, :fd], rt[:, :, :fd],
                                     GN[:, :, :fd])
                t2 = ew.tile([128, 2, CB], bf16, tag="t2")
                nc.vector.tensor_add(t2[:, :, :fd], t1[:, :, :fd],
                                     XN[:, :, :fd])
                nt = ew.tile([128, 2, CB], bf16, tag="nt")
                nc.scalar.activation(nt[:, :, :fd], t2[:, :, :fd], AF.Tanh)
                st = ew.tile([128, 2, CB], bf16, tag="st")
                nc.gpsimd.tensor_add(st[:, :, :fd], pT_bf[:, :, asl],
                                     pT_bf[:, :, lsl])
                dt_ = ew.tile([128, 2, CB], bf16, tag="dt")
                nc.vector.scalar_tensor_tensor(
                    dt_[:, :, :fd], st[:, :, :fd], 0.5, nt[:, :, :fd],
                    ALU.mult, ALU.subtract)
                et = ew.tile([128, 2, CB], bf16, tag="et")
                nc.vector.tensor_mul(et[:, :, :fd], zt[:, :, :fd],
                                     dt_[:, :, :fd])
                nc.gpsimd.tensor_add(cT_bf[:, :, wsl], et[:, :, :fd],
                                     nt[:, :, :fd])

            for _rep in range(REPEAT):
                for d, (ilo, ihi, cbase) in enumerate(TAB):
                    k = ihi - ilo + 1
                    c0 = 0
                    for cs in _chunk_sizes(k):
                        emit_chunk(d, ilo, cbase, c0, c0 + cs)
                        c0 += cs

            # --- head: partial logits for this core's direction ---
            hfin = []
            sl = (len(TAB) - 1) % 2
            for kc in (0, 1):
                t = hd.tile([128, B], f32, tag=f"hf{kc}")
                nc.scalar.copy(t, h_bf[sl][:, kc, S * B:(S + 1) * B])
                hfin.append(t)
            pl_t = ps.tile([128, 2, CB], f32, tag="R")
            pl = pl_t[:B, 0, :OUT_DIM]
            for c, t in enumerate(hfin):
                nc.tensor.matmul(pl, t, wo_sb[:, c * OUT_DIM:(c + 1) * OUT_DIM],
                                 start=(c == 0), stop=(c == 1))
            ot = hd.tile([B, OUT_DIM], f32, tag="ot")
            nc.scalar.copy(ot, pl)
            nc.sync.dma_start(out=out_d[:, :], in_=ot)

    nc.compile()
    return nc


_CACHE = {}


def get_nc():
    if "nc" not in _CACHE:
        _CACHE["nc"] = _build_nc()
    return _CACHE["nc"]


def make_in_maps(x, Wx, Uh, Uh2, b, W_out, b_out):
    x = np.asarray(x, np.float32)
    wms = make_weight_maps(Wx, Uh, Uh2, b, W_out, b_out)
    in_maps = []
    for c in range(N_CORES):
        a = c % 4
        hb = c // 4
        xc = x[hb * B:(hb + 1) * B]
        m = dict(wms[a])
        m["pt"] = make_pt(xc, a)
        in_maps.append(m)
    return in_maps


def combine(parts, b_out):
    """parts: (8, 128, 10) per-core partial logits -> (256, 10) log_softmax."""
    parts = np.asarray(parts, np.float32)
    halves = [parts[0] + parts[1] + parts[2] + parts[3],
              parts[4] + parts[5] + parts[6] + parts[7]]
    logits = np.concatenate(halves, axis=0) \
        + np.asarray(b_out, np.float32)[None, :]
    mx = logits.max(axis=-1, keepdims=True)
    lse = np.log(np.exp(logits - mx).sum(axis=-1, keepdims=True)) + mx
    return (logits - lse).astype(np.float32)


def kernel(x, Wx, Uh, Uh2, b, W_out, b_out):
    from concourse.bass_utils import run_bass_kernel_spmd
    nc = get_nc()
    in_maps = make_in_maps(x, Wx, Uh, Uh2, b, W_out, b_out)
    res = run_bass_kernel_spmd(nc, in_maps, list(range(N_CORES)))
    parts = np.stack([res.results[c]["out"] for c in range(N_CORES)], axis=0)
    return combine(parts, b_out)
